# revision 9
# baseline (speedup 1.0000x reference)
"""Child-Sum TreeLSTM (perfect binary tree, depth 14) on 8 Trainium2 NeuronCores.

Strategy (single SPMD kernel)
-----------------------------
Heap-order contiguous node sharding: core k owns nodes [k*n/8, (k+1)*n/8) of
every level lvl >= 3 (n = 2^lvl). Children of a core's node range at level lvl
are exactly its node range at level lvl+1, so levels 13..3 run with zero
cross-core communication. At the level-3 boundary an in-kernel AllGather
shares the 8 per-core (h, c) states; all cores then redundantly compute the
top 7 nodes (levels 2..0) and core 0's root output is used. A dummy AllGather
fires at kernel start to absorb the one-time collective-firmware warmup
(~60us) while the dense phase runs.

Within a core, state is transposed: [mem_dim(1024) partitions x n nodes], one
SBUF tile [128, 8*n] per level (M-tile m of the mem dim = col block m). Per
level, gate pre-activations accumulate in PSUM: 8 K-chunk matmuls against Wh
plus the x-projection. For big levels (12..10) the x-projection GEMM (3
K-chunks against Wx) is folded directly into the same PSUM accumulation; for
small levels (9..3) and top levels (2..0) the x-projections are precomputed
(unbiased) into resident SBUF during the upfront dense pass and added via an
identity matmul. Biases are applied by the gate activations. All matmul
inputs bf16 (fp32 PSUM accumulation); stored states bf16.
"""

import numpy as np
import ml_dtypes
from contextlib import ExitStack

import concourse.bass as bass
import concourse.tile as tile
from concourse import bacc, mybir
from concourse.bass_utils import run_bass_kernel_spmd

BF16 = ml_dtypes.bfloat16
P = 128
MEM = 1024
IN = 300
DEPTH = 14
NCORE = 8
MT = MEM // P  # 8 M-tiles of the mem dim

NX = 2047  # per-core nodes, levels 13..3
NTOP = 7  # top nodes (levels 2..0)
NXT = NX + NTOP  # 2054
NSM = 127  # small-level nodes (levels 9..3), kept SBUF-resident
SM0 = 1920  # first small-level node col

GATES = "ifou"
SIG = mybir.ActivationFunctionType.Sigmoid
TANH = mybir.ActivationFunctionType.Tanh
IDENT = mybir.ActivationFunctionType.Identity
ACT_FN = {"i": SIG, "f": SIG, "o": SIG, "u": TANH}

# per-core column offset of level lvl within the node axis (levels 13..3)
OFF = {13: 0}
for _lvl in range(12, 2, -1):
    OFF[_lvl] = OFF[_lvl + 1] + (2 ** (_lvl + 1)) // NCORE

LEAF_CHUNKS = [(0, 512), (512, 512)]  # lvl 13, fused, i/o/u only
SMALL_CHUNK = (SM0, NSM)  # lvl 9..3 -> resident SBUF
TOP_CHUNK = (NX, NTOP)  # lvl 2..0 -> resident SBUF

F32 = mybir.dt.float32
BF = mybir.dt.bfloat16

TOFF = {2: 0, 1: 4, 0: 6}  # offsets of level lvl inside the top-7 block


def _emit_xproj_and_leaf(nc, xt, bias, wxsb, xres, xtop_sb, pools):
    """Upfront dense pass: leaf level (13) gates+combine, plus unbiased
    x-projections for small levels (-> xres) and top levels (-> xtop_sb),
    (g,m)-major: col (gi*8+m)*NSM + ... Returns leaf state tiles h13, c13
    [128, 8*1024] (bf16)."""
    xpp, gp, hp, psum = pools
    h13 = hp.tile([P, MT * 1024], BF, tag="h_odd", name="h13")
    c13 = hp.tile([P, MT * 1024], BF, tag="c_odd", name="c13")
    for m in range(MT):
        leaf = {}
        for g in "iouf":
            gi = GATES.index(g)
            chunks = (LEAF_CHUNKS if g != "f" else []) + [SMALL_CHUNK, TOP_CHUNK]
            for c0, ncol in chunks:
                ps = psum.tile([P, ncol], F32, tag="ps", bufs=8, name=f"ps_{m}_{g}{c0}")
                for kx in range(3):
                    nc.tensor.matmul(
                        ps[:],
                        wxsb[:, (gi * 3 + kx) * MEM + m * P : (gi * 3 + kx) * MEM + (m + 1) * P],
                        xt[:, kx * NXT + c0 : kx * NXT + c0 + ncol],
                        start=(kx == 0),
                        stop=(kx == 2),
                    )
                if (c0, ncol) in LEAF_CHUNKS:
                    bias_ap = bias[:, gi * MT + m : gi * MT + m + 1]
                    tg = gp.tile([P, ncol], F32, tag="g", bufs=12, name=f"lf_{m}_{g}{c0}")
                    nc.scalar.activation(tg[:], ps[:], ACT_FN[g], bias=bias_ap)
                    leaf[(g, c0)] = tg
                elif (c0, ncol) == SMALL_CHUNK:
                    nc.scalar.activation(
                        xres[:, (gi * MT + m) * NSM : (gi * MT + m + 1) * NSM], ps[:], IDENT
                    )
                else:
                    nc.scalar.activation(
                        xtop_sb[:, (gi * MT + m) * NTOP : (gi * MT + m + 1) * NTOP], ps[:], IDENT
                    )
        # leaf combine: c = i*u ; h = o*tanh(c)
        for c0, ncol in LEAF_CHUNKS:
            cs = c13[:, m * 1024 + c0 : m * 1024 + c0 + ncol]
            nc.vector.tensor_mul(cs, leaf[("i", c0)][:], leaf[("u", c0)][:])
            th = gp.tile([P, ncol], F32, tag="g", bufs=12, name=f"th13_{m}_{c0}")
            nc.scalar.activation(th[:], cs, TANH)
            nc.vector.tensor_mul(h13[:, m * 1024 + c0 : m * 1024 + c0 + ncol], leaf[("o", c0)][:], th[:])
    return h13, c13


def _emit_level(nc, wh, ident, bias, h_ch, c_ch, n, x_add, par, pools, lvl, fused_f):
    """One non-leaf level. h_ch/c_ch: single tiles [128, 8*2n] bf16.
    x_add(gi, m, ps, first): emit the x-projection contribution (unbiased)
    into PSUM tile ps (matmul accumulation; first=True -> start group).
    Biases are applied by the gate activations here.
    Returns (h_new, c_new) single tiles [128, 8*n] bf16."""
    xpp, gp, hp, psum = pools
    n2 = 2 * n

    def wh_ap(gi, k, m):
        return wh[:, (gi * MT + k) * MEM + m * P : (gi * MT + k) * MEM + (m + 1) * P]

    def bias_ap(gi, m):
        return bias[:, gi * MT + m : gi * MT + m + 1]

    h_new = hp.tile([P, MT * n], BF, tag=f"h_{'odd' if par else 'even'}", name=f"h_{lvl}")
    c_new = hp.tile([P, MT * n], BF, tag=f"c_{'odd' if par else 'even'}", name=f"c_{lvl}")

    # fused-f matmuls first: they depend only on child h, so the PE has dense
    # work at level start while the DVE computes hsum.
    ff_of = {}
    gi_f = GATES.index("f")
    if fused_f:
        for m in range(MT):
            psf = psum.tile([P, n2], F32, tag="ps", bufs=8, name=f"psf_{lvl}_{m}")
            for k in range(MT):
                nc.tensor.matmul(
                    psf[:], wh_ap(gi_f, k, m), h_ch[:, k * n2 : (k + 1) * n2], start=(k == 0), stop=False
                )
            x_add(gi_f, m, psf, True)
            ff = gp.tile([P, n2], F32, tag="g", bufs=12, name=f"ff_{lvl}_{m}")
            nc.scalar.activation(ff[:], psf[:], SIG, bias=bias_ap(gi_f, m))
            ff_of[m] = ff

    hs = hp.tile([P, MT * n], BF, tag=f"hs_{par}", name=f"hs_{lvl}")
    for k in range(MT):
        nc.vector.tensor_add(
            hs[:, k * n : (k + 1) * n], h_ch[:, k * n2 : (k + 1) * n2 : 2], h_ch[:, k * n2 + 1 : (k + 1) * n2 : 2]
        )

    for m in range(MT):
        gio = {}
        for g in "iou":
            gi = GATES.index(g)
            ps = psum.tile([P, n], F32, tag="ps", bufs=8, name=f"ps_{lvl}_{m}{g}")
            for k in range(MT):
                nc.tensor.matmul(ps[:], wh_ap(gi, k, m), hs[:, k * n : (k + 1) * n], start=(k == 0), stop=False)
            x_add(gi, m, ps, False)
            tg = gp.tile([P, n], F32, tag="g", bufs=12, name=f"t{g}_{lvl}_{m}")
            nc.scalar.activation(tg[:], ps[:], ACT_FN[g], bias=bias_ap(gi, m))
            gio[g] = tg[:]

        if fused_f:
            prod = gp.tile([P, n2], F32, tag="g", bufs=12, name=f"prod_{lvl}_{m}")
            nc.vector.tensor_mul(prod[:], ff_of[m][:], c_ch[:, m * n2 : (m + 1) * n2])
            fc = gp.tile([P, n], F32, tag="g", bufs=12, name=f"fc_{lvl}_{m}")
            nc.vector.tensor_add(fc[:], prod[:, 0:n2:2], prod[:, 1:n2:2])
        else:
            psL = psum.tile([P, n], F32, tag="ps", bufs=8, name=f"psL_{lvl}_{m}")
            psR = psum.tile([P, n], F32, tag="ps", bufs=8, name=f"psR_{lvl}_{m}")
            for k in range(MT):
                w = wh_ap(gi_f, k, m)
                nc.tensor.matmul(psL[:], w, h_ch[:, k * n2 : (k + 1) * n2 : 2], start=(k == 0), stop=False)
                nc.tensor.matmul(psR[:], w, h_ch[:, k * n2 + 1 : (k + 1) * n2 : 2], start=(k == 0), stop=False)
            x_add(gi_f, m, psL, False)
            x_add(gi_f, m, psR, False)
            fL = gp.tile([P, n], F32, tag="g", bufs=12, name=f"fL_{lvl}_{m}")
            nc.scalar.activation(fL[:], psL[:], SIG, bias=bias_ap(gi_f, m))
            fR = gp.tile([P, n], F32, tag="g", bufs=12, name=f"fR_{lvl}_{m}")
            nc.scalar.activation(fR[:], psR[:], SIG, bias=bias_ap(gi_f, m))
            t1 = gp.tile([P, n], F32, tag="g", bufs=12, name=f"t1_{lvl}_{m}")
            nc.vector.tensor_mul(t1[:], fL[:], c_ch[:, m * n2 : (m + 1) * n2 : 2])
            t2 = gp.tile([P, n], F32, tag="g", bufs=12, name=f"t2_{lvl}_{m}")
            nc.vector.tensor_mul(t2[:], fR[:], c_ch[:, m * n2 + 1 : (m + 1) * n2 : 2])
            fc = gp.tile([P, n], F32, tag="g", bufs=12, name=f"fc_{lvl}_{m}")
            nc.vector.tensor_add(fc[:], t1[:], t2[:])

        tiu = gp.tile([P, n], F32, tag="g", bufs=12, name=f"tiu_{lvl}_{m}")
        nc.vector.tensor_mul(tiu[:], gio["i"], gio["u"])
        cm = c_new[:, m * n : (m + 1) * n]
        nc.vector.tensor_add(cm, tiu[:], fc[:])
        th = gp.tile([P, n], F32, tag="g", bufs=12, name=f"th_{lvl}_{m}")
        nc.scalar.activation(th[:], cm, TANH)
        nc.vector.tensor_mul(h_new[:, m * n : (m + 1) * n], gio["o"], th[:])
    return h_new, c_new


def build_kernel():
    nc = bacc.Bacc("TRN2", target_bir_lowering=False, debug=False, num_devices=NCORE)
    xT_d = nc.dram_tensor("xT", [P, 3 * NXT], BF, kind="ExternalInput").ap()
    wxT_d = nc.dram_tensor("wxT", [P, 4 * 3 * MEM], BF, kind="ExternalInput").ap()
    whT_d = nc.dram_tensor("whT", [P, 4 * MT * MEM], BF, kind="ExternalInput").ap()
    bias_d = nc.dram_tensor("bias", [P, 32], F32, kind="ExternalInput").ap()
    ident_d = nc.dram_tensor("ident", [P, P], BF, kind="ExternalInput").ap()
    root_d = nc.dram_tensor("root", [P, 16], F32, kind="ExternalOutput").ap()

    with tile.TileContext(nc) as tc, ExitStack() as ctx:
        const = ctx.enter_context(tc.tile_pool(name="const", bufs=1))
        xpp = ctx.enter_context(tc.tile_pool(name="xpp", bufs=8))
        gp = ctx.enter_context(tc.tile_pool(name="gp", bufs=12))
        hp = ctx.enter_context(tc.tile_pool(name="hp", bufs=1))
        psum = ctx.enter_context(tc.tile_pool(name="psum", bufs=8, space="PSUM"))
        dram = ctx.enter_context(tc.tile_pool(name="dram", bufs=1, space="DRAM"))
        pools = (xpp, gp, hp, psum)

        ident = const.tile([P, P], BF, name="ident_sb")
        nc.sync.dma_start(ident[:], ident_d[:])

        # Dummy warmup collective: absorbs the one-time ncfw/collective path
        # startup (~60us) in the background while the dense phase runs.
        warm_in = dram.tile([P, 1], BF, name="warm_in")
        warm_out = dram.tile([NCORE * P, 1], BF, name="warm_out")
        nc.gpsimd.dma_start(warm_in[:], ident[:, 0:1])
        nc.gpsimd.collective_compute(
            "AllGather",
            mybir.AluOpType.bypass,
            replica_groups=[list(range(NCORE))],
            ins=[warm_in.opt()],
            outs=[warm_out.opt()],
        )

        xt = const.tile([P, 3 * NXT], BF, name="xt")
        nc.sync.dma_start(xt[:], xT_d[:])
        bias = const.tile([P, 32], F32, name="bias_sb")
        nc.sync.dma_start(bias[:], bias_d[:])
        wxsb = const.tile([P, 4 * 3 * MEM], BF, name="wxsb")
        nc.sync.dma_start(wxsb[:], wxT_d[:])
        xres = const.tile([P, 32 * NSM], BF, name="xres")
        xtop_sb = const.tile([P, 32 * NTOP], BF, name="xtop_sb")

        h_ch, c_ch = _emit_xproj_and_leaf(nc, xt, bias, wxsb, xres, xtop_sb, pools)

        # weights for the h-GEMMs load during the xproj/leaf pass
        wh = const.tile([P, 4 * MT * MEM], BF, name="wh_sb")
        for gi in range(4):
            s = gi * MT * MEM
            nc.sync.dma_start(wh[:, s : s + MT * MEM], whT_d[:, s : s + MT * MEM])

        for lvl in range(12, 2, -1):
            n = 1 << (lvl - 3)
            par = lvl & 1
            # fused-f needs duplicated x, only available via resident xres
            fused_f = lvl <= 9
            if lvl >= 10:
                # fold the x-projection GEMM into the level PSUM accumulation
                off = OFF[lvl]

                def x_add(gi, m, ps, dup, off=off, n=n):
                    assert not dup
                    for kx in range(3):
                        nc.tensor.matmul(
                            ps[:],
                            wxsb[:, (gi * 3 + kx) * MEM + m * P : (gi * 3 + kx) * MEM + (m + 1) * P],
                            xt[:, kx * NXT + off : kx * NXT + off + n],
                            start=False,
                            stop=(kx == 2),
                        )

            else:
                off = OFF[lvl] - SM0

                def x_add(gi, m, ps, dup, off=off, n=n, lvl=lvl):
                    if dup:
                        xpf = xres[:, (gi * MT + m) * NSM + off : (gi * MT + m) * NSM + off + n]
                        xpf2 = xpp.tile([P, 2 * n], BF, tag="xpf2", bufs=4, name=f"xpf2_{lvl}_{m}")
                        nc.vector.tensor_copy(xpf2[:, 0 : 2 * n : 2], xpf)
                        nc.vector.tensor_copy(xpf2[:, 1 : 2 * n : 2], xpf)
                        nc.tensor.matmul(ps[:], ident[:], xpf2[:], start=False, stop=True)
                    else:
                        nc.tensor.matmul(
                            ps[:],
                            ident[:],
                            xres[:, (gi * MT + m) * NSM + off : (gi * MT + m) * NSM + off + n],
                            start=False,
                            stop=True,
                        )

            h_ch, c_ch = _emit_level(nc, wh, ident, bias, h_ch, c_ch, n, x_add, par, pools, lvl, fused_f)

        # ---- AllGather the 8 per-core (h3, c3) states ----
        out16 = gp.tile([P, 16], BF, tag="g", bufs=12, name="out16")
        nc.vector.tensor_copy(out16[:, 0:8], h_ch[:])
        nc.vector.tensor_copy(out16[:, 8:16], c_ch[:])
        ag_in = dram.tile([P, 16], BF, name="ag_in")
        ag_out = dram.tile([NCORE * P, 16], BF, name="ag_out")
        nc.gpsimd.dma_start(ag_in[:], out16[:])
        nc.gpsimd.collective_compute(
            "AllGather",
            mybir.AluOpType.bypass,
            replica_groups=[list(range(NCORE))],
            ins=[ag_in.opt()],
            outs=[ag_out.opt()],
        )
        # gathered layout: rank j block [128, 16]: cols 0..7 h m-tiles, 8..15 c
        hc_tmp = const.tile([P, NCORE * 16], BF, name="hc_tmp")
        nc.sync.dma_start(
            hc_tmp[:].rearrange("p (j c) -> p j c", c=16),
            ag_out[:].rearrange("(j p) c -> p j c", j=NCORE),
        )
        h3sb = const.tile([P, MT * NCORE], BF, name="h3sb")
        c3sb = const.tile([P, MT * NCORE], BF, name="c3sb")
        hc_view = hc_tmp[:].rearrange("p (j c) -> p c j", c=16)
        nc.vector.tensor_copy(h3sb[:].rearrange("p (m j) -> p m j", j=NCORE), hc_view[:, 0:8, :])
        nc.vector.tensor_copy(c3sb[:].rearrange("p (m j) -> p m j", j=NCORE), hc_view[:, 8:16, :])

        # ---- top levels 2..0, computed redundantly on every core ----
        h_ch, c_ch = h3sb, c3sb
        for lvl in range(2, -1, -1):
            n = 1 << lvl
            toff = TOFF[lvl]

            def x_add(gi, m, ps, dup, toff=toff, n=n):
                assert not dup
                nc.tensor.matmul(
                    ps[:],
                    ident[:],
                    xtop_sb[:, (gi * MT + m) * NTOP + toff : (gi * MT + m) * NTOP + toff + n],
                    start=False,
                    stop=True,
                )

            h_ch, c_ch = _emit_level(
                nc, wh, ident, bias, h_ch, c_ch, n, x_add, lvl & 1, pools, lvl, False
            )

        out32 = gp.tile([P, 16], F32, tag="g", bufs=12, name="out32")
        nc.vector.tensor_copy(out32[:, 0:8], c_ch[:])
        nc.vector.tensor_copy(out32[:, 8:16], h_ch[:])
        nc.sync.dma_start(root_d[:], out32[:])
    nc.compile()
    return nc


_CACHE = {}


def _get_programs():
    if "a" not in _CACHE:
        _CACHE["a"] = build_kernel()
    return _CACHE["a"]


def _prep_host_inputs(embs, Ws, bs):
    wxT = np.zeros((P, 4 * 3 * MEM), BF16)
    whT = np.zeros((P, 4 * MT * MEM), BF16)
    bias = np.zeros((P, 32), np.float32)
    for gi, g in enumerate(GATES):
        WxT = Ws[g + "x"].T.astype(BF16)  # [300, 1024]
        for kx in range(3):
            rows = WxT[kx * P : (kx + 1) * P]
            wxT[: rows.shape[0], (gi * 3 + kx) * MEM : (gi * 3 + kx + 1) * MEM] = rows
        WhT = Ws[g + "h"].T.astype(BF16)  # [1024, 1024]
        for k in range(MT):
            whT[:, (gi * MT + k) * MEM : (gi * MT + k + 1) * MEM] = WhT[k * P : (k + 1) * P]
        bias[:, gi * MT : (gi + 1) * MT] = bs[g].reshape(MT, P).T
    ident = np.eye(P, dtype=BF16)

    x_top = embs[[3, 4, 5, 6, 1, 2, 0]].T  # [300, 7], heap order per level
    in_maps = []
    for k in range(NCORE):
        cols = []
        for lvl in range(DEPTH - 1, 2, -1):
            n = 1 << lvl
            nl = n // NCORE
            cols.append(embs[n - 1 + k * nl : n - 1 + (k + 1) * nl].T)
        x_all = np.concatenate(cols + [x_top], axis=1)  # [300, 2054]
        xT = np.zeros((P, 3 * NXT), BF16)
        for kx in range(3):
            rows = x_all[kx * P : (kx + 1) * P].astype(BF16)
            xT[: rows.shape[0], kx * NXT : (kx + 1) * NXT] = rows
        in_maps.append({"xT": xT, "wxT": wxT, "whT": whT, "bias": bias, "ident": ident})
    return in_maps


def kernel(**inputs):
    embs = np.asarray(inputs["embs"], dtype=np.float32)
    depth = int(np.asarray(inputs["depth"]))
    assert depth == DEPTH and embs.shape == (2**DEPTH - 1, IN)
    Ws = {g + s: np.asarray(inputs["W" + g + s], dtype=np.float32) for g in GATES for s in "xh"}
    bs = {g: np.asarray(inputs["b" + g + "x"]) + np.asarray(inputs["b" + g + "h"]) for g in GATES}

    nc_a = _get_programs()
    in_maps = _prep_host_inputs(embs, Ws, bs)
    res = run_bass_kernel_spmd(nc_a, in_maps, core_ids=list(range(NCORE))).results

    root = res[0]["root"]  # [128, 16] f32: cols 0..7 = c M-tiles, 8..15 = h
    c_root = root[:, :8].T.reshape(MEM)
    h_root = root[:, 8:].T.reshape(MEM)
    return np.stack([c_root, h_root]).astype(np.float32)
